# revision 8
# baseline (speedup 1.0000x reference)
"""DTW (dynamic time warping) distance kernel for Trainium2, 8-core SPMD.

Problem: B=32 independent (x[b] in R^{1024x64}, y[b] in R^{1024x64}) pairs.
For each pair: dist = cdist(x, y) (euclidean, [1024, 1024]); DTW dynamic
program over dist; output D[N, M] scalar per pair.

Sharding: embarrassingly parallel over batch. 8 cores x 4 batches each.

Per-core algorithm:
  Phase 1 (cdist): dist^2 = xsq_i + ysq_j - 2 x.y^T via one augmented
  matmul per [128, 512] tile (K=65: 64 feature rows of -2*x^T plus a ones
  row pairing with a ysq row); xsq added as the ACT bias of the Relu pass;
  then Sqrt. Tiles are DMAed to an internal DRAM buffer in the "DP layout"
  the DTW scan wants: [b, q, i, r] with j = 32q + r.

  Phase 2 (DTW): row recurrence cur[j] = cost[j] + min(prev[j], prev[j-1],
  cur[j-1]) vectorized via tensor_tensor_scan. Layout: partition p = 32b+q
  (batch b, column-chunk q of 32), free r = within-chunk column. Per row:
  chunk-local scans (A with init inf, B = local cumsum), chunk-end values
  hoisted to "row space" with one 32x32 stream-transpose per tile, a single
  strided-partition scan combines the 32 chunks per batch, and one
  scalar_tensor_tensor applies x = min(B + u_in, A). Cross-chunk shift for
  the diagonal term uses stream_shuffle.
"""

import numpy as np

import concourse.bass as bass
import concourse.bacc as bacc
import concourse.mybir as mybir
from concourse.tile import TileContext
from concourse.masks import make_identity
from concourse import bass_utils

f32 = mybir.dt.float32
ADD = mybir.AluOpType.add
MIN = mybir.AluOpType.min
MAX = mybir.AluOpType.max
MULT = mybir.AluOpType.mult
ACT = mybir.ActivationFunctionType

N_CORES = 8
NB = 4          # batches per core
N = 1024        # rows (x length)
M = 1024        # cols (y length)
F = 64          # features
NQ = 32         # column chunks
CL = 32         # chunk length (NQ*CL == M)
BIG = 3.0e38    # finite stand-in for +inf
SHIFT1 = [0] + list(range(31))  # stream_shuffle: out[m] = in[m-1] per 32-block


def _emit_cdist(nc, sb, ps, psmm, x_d, y_d, dist_tiles, n_rows):
    """Emit phase 1. dist_tiles: list of DRAM tiles [NB, NQ, 128, CL], one
    per 128-row block of the DP."""
    n_ti = n_rows // 128

    ident = sb.tile([128, 128], f32)
    make_identity(nc, ident[:])
    ones64 = sb.tile([64, 1], f32)
    nc.vector.memset(ones64[:], 1.0)

    XTA, YTA, XSQ = [], [], []
    for b in range(NB):
        XN = sb.tile([128, 8 * F], f32, tag="XN", bufs=2)
        YN = sb.tile([128, 8 * F], f32, tag="YN", bufs=2)
        xta = sb.tile([65, N], f32, tag=f"XTA{b}")
        yta = sb.tile([65, M], f32, tag=f"YTA{b}")
        xsq = sb.tile([128, 8], f32, tag=f"XSQ{b}")
        ysqel = sb.tile([64, M], f32, tag="YSQel", bufs=2)
        sqs = sb.tile([128, F], f32, tag="sqs", bufs=2)

        # natural-layout loads: partition = i%128, free = (i//128, f).
        # SWDGE (gpsimd) keeps these on one queue -> one wait at consumers.
        nc.gpsimd.dma_start(
            XN[:], bass.AP(x_d, b * N * F, [[F, 128], [128 * F, 8], [1, F]])
        )
        nc.gpsimd.dma_start(
            YN[:], bass.AP(y_d, b * M * F, [[F, 128], [128 * F, 8], [1, F]])
        )

        # PE transposes -> feature-major; x scaled by -2 on the PSUM copy-out.
        # 4 transposes share one [64, 512] PSUM tile so each 512-col stripe of
        # XTA/YTA has a single ACT producer (keeps matmul wait counts low).
        for g in range(2):
            pt = ps.tile([64, 512], f32, tag="pt")
            for tt in range(4):
                t = 4 * g + tt
                nc.tensor.transpose(
                    pt[:, tt * 128 : (tt + 1) * 128],
                    YN[:, t * F : (t + 1) * F], ident[:],
                )
            nc.scalar.activation(yta[0:64, g * 512 : (g + 1) * 512], pt[:], ACT.Copy)
        for g in range(max(1, n_ti // 4)):
            pt = ps.tile([64, 512], f32, tag="pt")
            nt = min(4, n_ti - 4 * g)
            for tt in range(nt):
                t = 4 * g + tt
                nc.tensor.transpose(
                    pt[:, tt * 128 : (tt + 1) * 128],
                    XN[:, t * F : (t + 1) * F], ident[:],
                )
            nc.scalar.activation(
                xta[0:64, g * 512 : g * 512 + nt * 128],
                pt[:, 0 : nt * 128], ACT.Copy, scale=-2.0,
            )
        # xsq[i] per i-tile column (ACT Square with accumulate)
        for t in range(n_ti):
            nc.scalar.activation(
                sqs[:], XN[:, t * F : (t + 1) * F], ACT.Square,
                accum_out=xsq[:, t : t + 1],
            )
        # augmented rows: xta row 64 = ones; yta row 64 = ysq
        nc.vector.memset(xta[64:65, :], 1.0)
        nc.gpsimd.tensor_tensor(ysqel[:], yta[0:64, :], yta[0:64, :], MULT)
        for nj in range(2):
            py = ps.tile([1, 512], f32, tag="py")
            nc.tensor.matmul(
                py[:], ones64[:], ysqel[:, nj * 512 : (nj + 1) * 512],
                start=True, stop=True,
            )
            nc.scalar.activation(
                yta[64:65, nj * 512 : (nj + 1) * 512], py[:], ACT.Copy
            )
        XTA.append(xta)
        YTA.append(yta)
        XSQ.append(xsq)

    # dist tiles: matmul + relu(+xsq bias) + sqrt + DMA out in DP layout.
    # One DMA-out per (ti, b) from a consolidated [128, 1024] tile keeps the
    # producer count of each dist_tiles[ti] low (4 DMAs).
    for ti in range(n_ti):
        for b in range(NB):
            ds2 = sb.tile([128, 1024], f32, tag="DS2", bufs=2)
            for nj in range(2):
                pq = psmm.tile([128, 512], f32, tag="pq")
                nc.tensor.matmul(
                    pq[:],
                    XTA[b][:, ti * 128 : (ti + 1) * 128],
                    YTA[b][:, nj * 512 : (nj + 1) * 512],
                    start=True, stop=True,
                )
                ds = sb.tile([128, 512], f32, tag="DS", bufs=3)
                nc.scalar.activation(
                    ds[:], pq[:], ACT.Relu, bias=XSQ[b][:, ti : ti + 1]
                )
                nc.scalar.activation(
                    ds2[:, nj * 512 : (nj + 1) * 512], ds[:], ACT.Sqrt
                )
            # -> dist_tiles[ti][b, q, p, r],  j = 32q + r
            dst = bass.AP(
                dist_tiles[ti].tensor,
                b * NQ * 128 * CL,
                [[CL, 128], [128 * CL, NQ], [1, CL]],
            )
            src = bass.AP(ds2.tensor, 0, [[1024, 128], [CL, NQ], [1, CL]])
            nc.sync.dma_start(dst, src)


def _emit_dtw(nc, sb, dist_tiles, xout_d, n_rows):
    """Emit phase 2: the sequential DP over n_rows rows."""
    n_ti = n_rows // 128
    RW = 128  # rows per ring half

    ring = sb.tile([128, 2 * RW * CL], f32)
    X = sb.tile([128, CL], f32)
    Mn = sb.tile([128, CL], f32)
    AB = sb.tile([128, 96], f32)
    TPA = sb.tile([128, 32], f32)
    TPB = sb.tile([128, 32], f32)
    RBu = sb.tile([128, 33], f32)
    UC = sb.tile([128, 32], f32)
    ECOL = sb.tile([128, 1], f32)
    INJ = sb.tile([128, 1], f32)
    INFT = sb.tile([128, CL], f32)

    nc.vector.memset(INFT[:], BIG)
    nc.vector.memset(INJ[:], -BIG)
    for b in range(NB):
        nc.vector.memset(INJ[32 * b : 32 * b + 1, :], BIG)
    nc.vector.memset(RBu[:], BIG)       # col 0 must stay BIG (u-scan shift-in)
    nc.vector.memset(AB[:, 64:96], 0.0)  # pad read by end-col transpose views

    ring_pitch = 2 * RW * CL

    for blk in range(n_ti):
        # DMA 128 rows (all batches/chunks) into ring half blk%2
        half = (blk % 2) * RW * CL
        dst = bass.AP(
            ring.tensor, half, [[ring_pitch, 128], [CL, RW], [1, CL]]
        )
        src = bass.AP(
            dist_tiles[blk].tensor, 0,
            [[NQ * 128 * CL, NB], [128 * CL, NQ], [CL, RW], [1, CL]],
        )
        nc.sync.dma_start(dst, src)

        for ii in range(RW):
            i = blk * RW + ii
            CR = ring[:, half + ii * CL : half + (ii + 1) * CL]
            if i == 0:
                # first DP row: mins = [0, BIG, ...] (diag D[0,0] = 0)
                nc.vector.memset(Mn[:], BIG)
                for b in range(NB):
                    nc.vector.memset(Mn[32 * b : 32 * b + 1, 0:1], 0.0)
            else:
                nc.vector.stream_shuffle(ECOL[:], X[:, CL - 1 : CL], SHIFT1)
                nc.vector.scalar_tensor_tensor(
                    Mn[:, 0:1], ECOL[:], INJ[:, 0:1], X[:, 0:1], MAX, MIN
                )
                nc.vector.tensor_tensor(
                    Mn[:, 1:CL], X[:, 1:CL], X[:, 0 : CL - 1], MIN
                )
            # chunk-local scans: A (DP with init inf), B (local cumsum)
            nc.vector.tensor_tensor_scan(AB[:, 0:CL], Mn[:], CR, BIG, MIN, ADD)
            nc.vector.tensor_tensor_scan(
                AB[:, CL : 2 * CL], CR, INFT[:], 0.0, ADD, MIN
            )
            # end columns -> row space (one stream-transpose each)
            nc.vector.transpose(
                TPA[:], bass.AP(AB.tensor, CL - 1, [[96, 128], [0, 32]])
            )
            nc.vector.transpose(
                TPB[:], bass.AP(AB.tensor, 2 * CL - 1, [[96, 128], [0, 32]])
            )
            # cross-chunk combine: u = min(u + B_e, A_e), one scan per batch
            # (walrus rejects partition-strided APs; starts {0,32,64,96} ok)
            for b in range(NB):
                nc.vector.tensor_tensor_scan(
                    RBu[32 * b : 32 * b + 1, 1:33],
                    TPB[32 * b : 32 * b + 1, 0:32],
                    TPA[32 * b : 32 * b + 1, 0:32],
                    BIG, ADD, MIN,
                )
            # back to column space: UC[32b+q, 0] = u_shift
            nc.vector.transpose(UC[:], RBu[:, 0:32])
            # apply: X = min(B + u_in, A)
            nc.vector.scalar_tensor_tensor(
                X[:], AB[:, CL : 2 * CL], UC[:, 0:1], AB[:, 0:CL], ADD, MIN
            )

    nc.sync.dma_start(xout_d[:], X[:])


def build_nc(n_rows=N):
    nc = bacc.Bacc()
    x_d = nc.dram_tensor("x", [NB, N, F], f32, kind="ExternalInput")
    y_d = nc.dram_tensor("y", [NB, M, F], f32, kind="ExternalInput")
    xout_d = nc.dram_tensor("xout", [128, CL], f32, kind="ExternalOutput")

    n_ti = n_rows // 128
    with TileContext(nc) as tc:
        with (
            tc.tile_pool(name="sb", bufs=1) as sb,
            tc.tile_pool(name="ps", bufs=2, space="PSUM") as ps,
            tc.tile_pool(name="psmm", bufs=4, space="PSUM") as psmm,
            tc.tile_pool(name="dr", bufs=1, space="DRAM") as dr,
        ):
            dist_tiles = [
                dr.tile([NB, NQ, 128, CL], f32, name=f"distbuf{t}")
                for t in range(n_ti)
            ]
            _emit_cdist(nc, sb, ps, psmm, x_d, y_d, dist_tiles, n_rows)
            _emit_dtw(nc, sb, dist_tiles, xout_d, n_rows)
    nc.compile()
    return nc


_NC_CACHE = {}


def _get_nc(n_rows=N):
    if n_rows not in _NC_CACHE:
        _NC_CACHE[n_rows] = build_nc(n_rows)
    return _NC_CACHE[n_rows]


def kernel(x: np.ndarray, y: np.ndarray) -> np.ndarray:
    """x, y: [32, 1024, 64] float32 -> [32] float32 of DTW distances."""
    x = np.ascontiguousarray(x, dtype=np.float32)
    y = np.ascontiguousarray(y, dtype=np.float32)
    nc = _get_nc()
    in_maps = [
        {"x": x[NB * c : NB * (c + 1)], "y": y[NB * c : NB * (c + 1)]}
        for c in range(N_CORES)
    ]
    res = bass_utils.run_bass_kernel_spmd(nc, in_maps, core_ids=list(range(N_CORES)))
    out = np.empty((N_CORES * NB,), np.float32)
    for c in range(N_CORES):
        xo = res.results[c]["xout"]
        for b in range(NB):
            out[NB * c + b] = xo[32 * b + 31, CL - 1]
    return out
